# revision 1
# baseline (speedup 1.0000x reference)
"""DANet 3-layer MLP (B=8192, D=2048) on 8 Trainium2 NeuronCores.

Data-parallel: each core computes 1024 rows of the batch; the three
weight matrices are replicated. On-device everything lives in a
transposed layout (features on SBUF partitions) so the contraction dim
of every matmul is the partition dim and activations chain from layer
to layer without transposes; the host transposes x in and z/a out.

Matmuls run as float32r (TF32-like, full PE rate); PSUM accumulates in
fp32 over 16 k-tiles. Per output tile the DVE adds the bias for the z
output while the ACT engine computes tanh(psum + b) straight into the
next layer's activation buffer.
"""

import numpy as np

import concourse.mybir as mybir
import concourse.tile as tile
from concourse import bacc
from concourse.bass_utils import run_bass_kernel_spmd

NCORES = 8
B = 8192
D = 2048
BL = B // NCORES          # 1024 batch rows per core
P = 128                   # partitions
KT = D // P               # 16 contraction tiles
NPANEL = 512              # weight-panel width (n features per panel)
NPB = D // NPANEL         # 4 panels per layer
NSB = NPANEL // P         # 4 output-feature subblocks per panel
MBLK = 512                # moving-operand width (batch cols per matmul)
MT = BL // MBLK           # 2 batch blocks

f32 = mybir.dt.float32
f32r = mybir.dt.float32r
TANH = mybir.ActivationFunctionType.Tanh

W_BUFS = 24               # weight pool slots ([128,512] each, 2KB/partition)


def build_nc():
    nc = bacc.Bacc()

    xT = nc.dram_tensor("xT", [D, BL], f32, kind="ExternalInput")
    Ws = [nc.dram_tensor(f"W{l}", [D, D], f32, kind="ExternalInput")
          for l in range(3)]
    bs = [nc.dram_tensor(f"b{l}", [D], f32, kind="ExternalInput")
          for l in range(3)]
    zouts = [nc.dram_tensor(f"z{l}T", [D, BL], f32, kind="ExternalOutput")
             for l in range(3)]
    aouts = [nc.dram_tensor(f"a{l}T", [D, BL], f32, kind="ExternalOutput")
             for l in range(3)]

    with tile.TileContext(nc) as tc:
        with (
            tc.tile_pool(name="acts", bufs=1) as actp,
            tc.tile_pool(name="wpool", bufs=W_BUFS) as wpool,
            tc.tile_pool(name="zpool", bufs=4) as zpool,
            tc.tile_pool(name="misc", bufs=1) as misc,
            tc.tile_pool(name="psum", bufs=4, space="PSUM") as psp,
        ):
            # Persistent ping-pong activation buffers, transposed layout:
            # acts[s][k] holds features [128k, 128k+128) x all 1024 batch cols.
            acts = [
                [actp.tile([P, BL], f32r, name=f"act{s}_{k}", tag=f"act{s}_{k}")
                 for k in range(KT)]
                for s in range(2)
            ]

            # All three biases in one [128, 48] tile; column l*16+c holds
            # b_l[128c : 128c+128].
            bias = misc.tile([P, 3 * KT], f32, name="bias", tag="bias")
            for l in range(3):
                nc.sync.dma_start(
                    bias[:, l * KT:(l + 1) * KT],
                    bs[l][:].rearrange("(c p) -> p c", p=P),
                )

            # x -> activation set 0
            for k in range(KT):
                nc.sync.dma_start(
                    acts[0][k][:], xT[k * P:(k + 1) * P, :].bitcast(f32r))

            for l in range(3):
                act_in = acts[l % 2]
                act_out = acts[(l + 1) % 2]
                for nb in range(NPB):
                    wts = []
                    for k in range(KT):
                        wt = wpool.tile([P, NPANEL], f32r,
                                        name=f"w_l{l}_p{nb}_k{k}", tag="wt")
                        nc.sync.dma_start(
                            wt[:],
                            Ws[l][k * P:(k + 1) * P,
                                  nb * NPANEL:(nb + 1) * NPANEL].bitcast(f32r),
                        )
                        wts.append(wt)
                    for ns in range(NSB):
                        ni = nb * NSB + ns          # output-feature block 0..15
                        bcol = bias[:, l * KT + ni:l * KT + ni + 1]
                        for m in range(MT):
                            ms = m * MBLK
                            psum = psp.tile([P, MBLK], f32,
                                            name=f"psum_{l}_{ni}_{m}", tag="psum")
                            for k in range(KT):
                                nc.tensor.matmul(
                                    psum[:],
                                    wts[k][:, ns * P:(ns + 1) * P],
                                    act_in[k][:, ms:ms + MBLK],
                                    start=(k == 0),
                                    stop=(k == KT - 1),
                                )
                            z_sb = zpool.tile([P, MBLK], f32,
                                              name=f"z_{l}_{ni}_{m}", tag="z_sb")
                            nc.vector.tensor_scalar_add(z_sb[:], psum[:], bcol)
                            nc.scalar.activation(
                                act_out[ni][:, ms:ms + MBLK], psum[:], TANH,
                                bias=bcol, scale=1.0,
                            )
                            nc.sync.dma_start(
                                zouts[l][ni * P:(ni + 1) * P, ms:ms + MBLK],
                                z_sb[:],
                            )
                        # both batch halves of act_out[ni] written -> store a
                        nc.sync.dma_start(
                            aouts[l][ni * P:(ni + 1) * P, :],
                            act_out[ni][:].bitcast(f32),
                        )

    nc.finalize()
    return nc


_NC_CACHE = None


def _get_nc():
    global _NC_CACHE
    if _NC_CACHE is None:
        _NC_CACHE = build_nc()
    return _NC_CACHE


def kernel(x, W0, b0, W1, b1, W2, b2):
    x = np.asarray(x, dtype=np.float32)
    weights = {
        "W0": np.asarray(W0, dtype=np.float32),
        "b0": np.asarray(b0, dtype=np.float32),
        "W1": np.asarray(W1, dtype=np.float32),
        "b1": np.asarray(b1, dtype=np.float32),
        "W2": np.asarray(W2, dtype=np.float32),
        "b2": np.asarray(b2, dtype=np.float32),
    }
    in_maps = []
    for c in range(NCORES):
        xT = np.ascontiguousarray(x[c * BL:(c + 1) * BL, :].T)
        in_maps.append({"xT": xT, **weights})

    res = run_bass_kernel_spmd(_get_nc(), in_maps, core_ids=list(range(NCORES)))

    out = np.empty((6, B, D), dtype=np.float32)
    for c in range(NCORES):
        r = res.results[c]
        rows = slice(c * BL, (c + 1) * BL)
        for l in range(3):
            out[l, rows, :] = r[f"z{l}T"].T
            out[3 + l, rows, :] = r[f"a{l}T"].T
    return out
